# revision 25
# baseline (speedup 1.0000x reference)
"""Distributed ring-attention kernel for Trainium2 (8 NeuronCores, Bass/Tile).

Strategy (seq-parallel attention, full softmax without max-subtraction):
  - Host: transpose/cast inputs to bf16; shard x.T column-wise (seq) across 8 cores.
  - Per core: project Q/K/V for its 512-seq shard; AllGather K^T and V
    across cores; compute full attention for its Q shard over the whole
    4096-length K/V; out-projection; write its y shard.
  - Collective schedule: ncfw boots ~21us in and runs a ~37us comm-init
    barrier, so no gathered data can land before ~85us. The K/V exchange is
    4 combined AllGathers of 2 head-pairs each ([kt | v] packed in one flat
    buffer per op), sized so the supply stays just ahead of the two exp
    engines' combined consumption (~0.22M elem/us).
  - Scores are computed transposed (S^T = K @ Q^T, kpos on partitions) so the
    exp'd probabilities feed the P@V matmul directly as the stationary-side
    contraction; head pairs ride the PE array row groups (tile_position) so
    the two 64-deep score matmuls run concurrently. Softmax denominator
    comes from a ones-column appended to V. Softmax skips max-subtraction:
    scores are O(1) here, so exp is numerically safe.
  - exp is SPLIT across two engines: ScalarE runs native exp on ~2/3 of the
    score groups; the DVE handles the rest with a Schraudolph-style bit
    trick: bf16_bits = int16(s * 128*log2e/8 + b), written through an int16
    bitcast of the bf16 tile (one tensor_scalar op per group). The constant
    bias of the trick cancels in softmax; only the sawtooth residual
    (~1.6e-2 worst case full-DVE, ~1.3e-2 at 1/3 share) remains.
"""

import numpy as np
import ml_dtypes

HID = 1024
HEADS = 16
HD = 64
S = 4096
NCORES = 8
SQ = S // NCORES          # 512 q rows per core
PAIRS = HEADS // 2        # 8 head pairs (128 rows of qkvT per pair)
KTILES = S // 128         # 32 kpos tiles per head
VAUG = HD + 1             # 65: V plus ones column
SCALE = 1.0 / np.sqrt(HD)

# Schraudolph exp in bf16-bit space: bits16 = trunc(a*s + b) approximates
# bf16(exp(s/8)) bit pattern.  a = 128*log2(e)*SCALE;  b = 128*(127-sigma)+0.5
SCH_A = 128.0 * 1.4426950408889634 * SCALE
SCH_B = 16245.48

_cache = {}


def _build():
    import concourse.bass as bass
    import concourse.mybir as mybir
    import concourse.tile as tile
    from concourse import bacc

    dt = mybir.dt
    nc = bacc.Bacc("TRN2", target_bir_lowering=False, debug=False,
                   num_devices=NCORES)

    xT = nc.dram_tensor("xT", [HID, SQ], dt.bfloat16, kind="ExternalInput").ap()
    wqkvT = nc.dram_tensor("wqkvT", [HID, 3 * HID], dt.bfloat16,
                           kind="ExternalInput").ap()
    woutT = nc.dram_tensor("woutT", [HID, HID], dt.bfloat16,
                           kind="ExternalInput").ap()
    y = nc.dram_tensor("y", [SQ, HID], dt.float32, kind="ExternalOutput").ap()

    with tile.TileContext(nc) as tc:
        _body(nc, tc, bass, mybir, xT, wqkvT, woutT, y)

    nc.compile()
    return nc


def _body(nc, tc, bass, mybir, xT, wqkvT, woutT, y):
    dt = mybir.dt
    f32, bf16 = dt.float32, dt.bfloat16
    RG = [list(range(NCORES))]

    with (
        tc.tile_pool(name="dram", bufs=1, space="DRAM") as dram,
        tc.tile_pool(name="resident", bufs=1) as res,
        tc.tile_pool(name="stream", bufs=1) as st,
    ):
        # ---- DRAM bounce buffers: one combined [kt... | v...] buffer per
        # UNIT of head pairs; first two units are single pairs so the
        # attention pipeline starts as early as possible, later units
        # amortize the ~16us fixed ncfw per-op cost.  V sections are staged
        # PRE-AUGMENTED ([q, hh, tl, 65] with the ones column included) so
        # the post-gather vah load is one contiguous-line DMA. ----
        UNITS = [[0], [1], [2, 3], [4, 5], [6, 7]]
        KSEC = 128 * SQ             # kt section elems, layout [r 128, q 512]
        VSEC = 128 * 2 * 4 * VAUG   # v section elems, layout [q, hh, tl, 65]
        unit_of = {}
        for u, prs in enumerate(UNITS):
            for i, p in enumerate(prs):
                unit_of[p] = (u, i)
        ktvb, ktvg = [], []
        for u, prs in enumerate(UNITS):
            usz = len(prs) * (KSEC + VSEC)
            ktvb.append(dram.tile([1, usz], bf16, name=f"ktvb{u}"))
            ktvg.append(dram.tile([NCORES, usz], bf16, addr_space="Shared",
                                  name=f"ktvg{u}"))

        # ---- load xT (hidden x local-seq), 8 resident tiles ----
        xt = []
        for k in range(8):
            t = res.tile([128, SQ], bf16, tag=f"xt{k}", name=f"xt{k}")
            nc.sync.dma_start(t[:], xT[k * 128:(k + 1) * 128, :])
            xt.append(t)

        # dummy partition_broadcast at boot: forces the gpsimd ucode library
        # load (~15us) NOW instead of mid-attention, where it head-of-line
        # blocked the Vector queue for ~30us behind the first normalize
        dl0 = st.tile([1, 16], f32, tag="dl0", bufs=1)
        nc.vector.memset(dl0[:], 1.0)
        dlb = st.tile([2, 16], f32, tag="dlb", bufs=1)
        nc.gpsimd.partition_broadcast(dlb[:], dl0[:])

        # wqkvT strip views for batched weight loads
        wq4 = wqkvT.rearrange("(k p) (m c) -> p m k c", p=128, c=128)
        wv2 = wqkvT.rearrange("(k p) (m c) -> p m k c", p=128, c=128)

        def kt_proj(m, psP):
            """K^T rows for pair m (qkvT rows 1024+m*128) -> its unit's
            bounce buffer."""
            u, i = unit_of[m]
            ws = st.tile([128, 8 * 128], bf16, tag="wl", bufs=4)
            nc.sync.dma_start(ws.rearrange("p (k c) -> p k c", c=128),
                              wq4[:, 8 + m, :, :])
            ps = psP.tile([128, SQ], f32, tag="proj", bufs=4)
            for k in range(8):
                nc.tensor.matmul(ps[:], ws[:, k * 128:(k + 1) * 128],
                                 xt[k][:], start=(k == 0), stop=(k == 7))
            sb = st.tile([128, SQ], bf16, tag="kt_stage", bufs=4)
            nc.vector.tensor_copy(sb[:], ps[:])
            dst = ktvb[u][0, i * KSEC:(i + 1) * KSEC]
            nc.sync.dma_start(dst.rearrange("(r q) -> r q", q=SQ), sb[:])

        def v_proj(u, psP):
            """V rows (natural [s, (i hh d)]) for unit u's pairs -> bounce.

            One psum group per s-tile of 128; output n*128 wide."""
            prs = UNITS[u]
            n = len(prs)
            wvs = st.tile([128, 8 * n * 128], bf16, tag="wvs", bufs=2)
            wvs3 = wvs.rearrange("p (k c) -> p k c", c=n * 128)
            nc.sync.dma_start(
                wvs3.rearrange("p k (pr c) -> p k pr c", c=128),
                wv2[:, 16 + prs[0]:16 + prs[0] + n, :, :].rearrange(
                    "p pr k c -> p k pr c"))
            vbase = n * KSEC
            for sti in range(4):
                ps = psP.tile([128, n * 128], f32, tag="proj", bufs=4)
                for k in range(8):
                    nc.tensor.matmul(
                        ps[:], xt[k][:, sti * 128:(sti + 1) * 128],
                        wvs[:, k * n * 128:(k + 1) * n * 128],
                        start=(k == 0), stop=(k == 7))
                # stage augmented [q, i, hh, 65] with the ones column so the
                # gathered buffer is directly vah-shaped
                sb = st.tile([128, n * 2 * VAUG], bf16, tag="kv_stage",
                             bufs=4)
                sb4 = sb.rearrange("q (i hh v) -> q i hh v", hh=2, v=VAUG)
                nc.vector.tensor_copy(
                    sb4[:, :, :, 0:HD],
                    ps.rearrange("q (i hh d) -> q i hh d", hh=2, d=HD))
                nc.vector.memset(sb4[:, :, :, HD], 1.0)
                for i in range(n):
                    vpart = ktvb[u][0, vbase + i * VSEC:
                                    vbase + (i + 1) * VSEC].rearrange(
                        "(q hh tl v) -> q hh tl v", q=128, hh=2, v=VAUG)
                    nc.sync.dma_start(vpart[:, :, sti, :], sb4[:, i, :, :])
            nc.gpsimd.collective_compute(
                "AllGather", mybir.AluOpType.bypass, replica_groups=RG,
                ins=[ktvb[u].opt()], outs=[ktvg[u].opt()])

        qt = [None] * PAIRS

        def q_proj(m, psP):
            ws = st.tile([128, 8 * 128], bf16, tag="wl", bufs=4)
            nc.sync.dma_start(ws.rearrange("p (k c) -> p k c", c=128),
                              wq4[:, m, :, :])
            ps = psP.tile([128, SQ], f32, tag="proj", bufs=4)
            for k in range(8):
                nc.tensor.matmul(ps[:], ws[:, k * 128:(k + 1) * 128],
                                 xt[k][:], start=(k == 0), stop=(k == 7))
            t = res.tile([128, SQ], bf16, tag=f"qt{m}", name=f"qt{m}")
            nc.vector.tensor_copy(t[:], ps[:])
            qt[m] = t

        with tc.tile_pool(name="psP", bufs=1, space="PSUM") as psP:
            # per unit: stage kt for both pairs, then v, then fire the
            # combined AllGather; collectives run serially on gpsimd so the
            # emission order is the wire order.  q_proj 2..7 are deferred
            # into the gather window to keep the PE warm (HAM throttle).
            for u in range(len(UNITS)):
                for p in UNITS[u]:
                    kt_proj(p, psP)
                v_proj(u, psP)
            q_proj(0, psP)
            q_proj(1, psP)

        # ---- attention (head pairs row-packed on the PE array) ----
        attn = []
        for p in range(PAIRS):
            t = res.tile([128, SQ], bf16, tag=f"attn{p}", name=f"attn{p}")
            attn.append(t)

        # out-projection weights (pair-stacked rows: odd heads at
        # partitions 64..127)
        wo5 = woutT.rearrange("(pp r) (o c) -> r o pp c", r=128, c=512)
        wo = []
        for och in range(2):
            w = res.tile([128, PAIRS * 512], bf16, tag=f"wo{och}",
                         name=f"wo{och}")
            nc.sync.dma_start(
                w.rearrange("r (pp c) -> r pp c", c=512), wo5[:, och])
            wo.append(w)

        # exp engine split: every 3rd 2-slot group goes to the DVE via the
        # Schraudolph bit trick (int16 write into the bf16 tile)
        gctr = [0]

        def emit_exp(pt, sc, gw):
            g = gctr[0]
            gctr[0] += 1
            if g % 3 == 2:
                nc.vector.tensor_scalar(
                    pt[:, 0:gw].bitcast(mybir.dt.int16), sc[:, 0:gw],
                    float(SCH_A), float(SCH_B),
                    mybir.AluOpType.mult, mybir.AluOpType.add)
            else:
                nc.scalar.activation(pt[:, 0:gw], sc[:, 0:gw],
                                     mybir.ActivationFunctionType.Exp,
                                     scale=float(SCALE))

        # deferred q projections run in the gather window (own pool scope,
        # sequential with psP/psA)
        with tc.tile_pool(name="psQ", bufs=1, space="PSUM") as psQ:
            for m in range(2, PAIRS):
                q_proj(m, psQ)

        def emit_loads(p):
            """Issue pair p's K^T strip + augmented-V loads from the
            gathered buffers (one contiguous-line DMA each).  Queue
            assignment alternates by pair parity so a prefetched load
            waiting on the next AllGather never head-of-line-blocks the
            previous pair's loads."""
            u, i = unit_of[p]
            n = len(UNITS[u])
            eng = nc.sync if p % 2 == 0 else nc.gpsimd
            ktg3 = ktvg[u][:, i * KSEC:(i + 1) * KSEC].rearrange(
                "c (r q) -> r c q", q=SQ)
            kth = st.tile([128, S], bf16, tag="kth", bufs=3)
            eng.dma_start(kth.rearrange("r (c q) -> r c q", q=SQ), ktg3)
            vbase = n * KSEC + i * VSEC
            vgv = ktvg[u][:, vbase:vbase + VSEC].rearrange(
                "c (q hh tv) -> hh q c tv", q=128, hh=2, tv=4 * VAUG)
            vah = []
            for e in range(2):
                va = st.tile([128, KTILES * VAUG], bf16, tag="vah", bufs=6)
                eng.dma_start(va.rearrange("q (c tv) -> q c tv",
                                           tv=4 * VAUG), vgv[e])
                vah.append(va)
            return kth, vah

        # prefetch depth 1: loads for pair p+1 are emitted at pair p's top.
        # Depth 2 head-of-line-blocked the DMA queues: a prefetched load
        # waiting on a not-yet-landed AllGather stalls every DMA behind it.
        #
        # The whole attention runs as ONE global stream of 2-slot groups
        # with the PV matmuls lagging RA groups behind the scores ACROSS
        # pair boundaries, so the PE never drains a PV-only flush tail
        # between pairs (that bubble cost ~3us x 8 pairs).
        RA = 5
        pending_norm = []
        loaded = {}
        GPP = KTILES * 2 // 2       # 32 groups per pair
        sched = [(p, g) for p in range(PAIRS) for g in range(GPP)]
        state = {}

        def emit_scores(p, g):
            if g == 0:
                if p + 1 < PAIRS:
                    loaded[p + 1] = emit_loads(p + 1)
                kth, vah = loaded.pop(p)
                pv = [psA.tile([128, 512], f32, tag="pv", bufs=2,
                               name=f"pv{p}_{e}") for e in range(2)]
                state[p] = (kth, vah, pv, [])
            kth, vah, pv, pts = state[p]
            sc = psA.tile([128, 1024], f32, tag="sc", bufs=3)
            for idx, e in enumerate((0, 1)):
                nc.tensor.matmul(
                    sc[:, idx * 512:(idx + 1) * 512],
                    kth[e * 64:(e + 1) * 64, g * 128:(g + 1) * 128],
                    qt[p][e * 64:(e + 1) * 64, :],
                    start=True, stop=True,
                    tile_position=(e * 64, 0))
            pt = st.tile([128, 1024], bf16, tag="pt", bufs=RA + 2)
            emit_exp(pt, sc, 1024)
            pts.append(pt)

        def emit_pv(p, g):
            kth, vah, pv, pts = state[p]
            pt = pts[g]
            for idx, e in enumerate((0, 1)):
                nc.tensor.matmul(
                    pv[e][0:VAUG, :],
                    vah[e][:, g * VAUG:(g + 1) * VAUG],
                    pt[:, idx * 512:(idx + 1) * 512],
                    start=(g == 0), stop=(g == KTILES - 1))
            if g == KTILES - 1:
                finish_pair(p)

        def finish_pair(p):
            # Evacuate pv to SBUF right away so the PSUM slots free for the
            # next pair; DEFER the divide chain's emission by one pair so
            # its cross-engine round trips (gpsimd broadcast -> DVE recip)
            # never sit at the head of the Vector queue blocking exps.
            kth, vah, pv, pts = state.pop(p)
            pvs2 = []
            for e in range(2):
                pvs = st.tile([VAUG, 512], f32, tag="pvs", bufs=4)
                nc.vector.tensor_copy(pvs[:], pv[e][0:VAUG, :])
                pvs2.append(pvs)

            def emit_norm(p=p, pvs2=pvs2):
                neng = nc.sync if p % 2 == 0 else nc.gpsimd
                for e in range(2):
                    pvs = pvs2[e]
                    l0 = st.tile([1, 512], f32, tag="l0", bufs=2)
                    neng.dma_start(l0[:], pvs[64:65, :])
                    lb = st.tile([64, 512], f32, tag="lb", bufs=2)
                    nc.gpsimd.partition_broadcast(lb[:], l0[:])
                    rb = st.tile([64, 512], f32, tag="rb", bufs=2)
                    nc.vector.reciprocal_approx_fast(rb[:], lb[:])
                    if e == 0:
                        nc.vector.tensor_mul(attn[p][0:64, :],
                                             pvs[0:64, :], rb[:])
                    else:
                        ao = st.tile([64, SQ], bf16, tag="ao", bufs=2)
                        nc.vector.tensor_mul(ao[:], pvs[0:64, :], rb[:])
                        neng.dma_start(attn[p][64:128, :], ao[:])

            if pending_norm:
                pending_norm.pop()()
            pending_norm.append(emit_norm)

        with tc.tile_pool(name="psA", bufs=1, space="PSUM") as psA:
            loaded[0] = emit_loads(0)
            for gi, (p, g) in enumerate(sched):
                emit_scores(p, g)
                if gi >= RA:
                    emit_pv(*sched[gi - RA])
            for gi in range(len(sched) - RA, len(sched)):
                emit_pv(*sched[gi])

        for fn in pending_norm:
            fn()
        pending_norm.clear()

        # ---- out projection: y[s, o] = sum_h attn_h^T.T @ woutT[h rows] ----
        with tc.tile_pool(name="psY", bufs=1, space="PSUM") as psY:
            for sti in range(4):
                for och in range(2):
                    psa = psY.tile([128, 512], f32, tag="ya", bufs=4)
                    for p in range(PAIRS):
                        nc.tensor.matmul(
                            psa[:], attn[p][:, sti * 128:(sti + 1) * 128],
                            wo[och][:, p * 512:(p + 1) * 512],
                            start=(p == 0), stop=(p == PAIRS - 1))
                    ysb = st.tile([128, 512], f32, tag="ysb", bufs=4)
                    nc.vector.tensor_copy(ysb[:], psa[:])
                    nc.sync.dma_start(
                        y[sti * 128:(sti + 1) * 128,
                          och * 512:(och + 1) * 512], ysb[:])


def _get_nc():
    if "nc" not in _cache:
        _cache["nc"] = _build()
    return _cache["nc"]


def kernel(x, W_qkv, W_out, _trace=False):
    from concourse.bass_utils import run_bass_kernel_spmd

    nc = _get_nc()
    bf16 = ml_dtypes.bfloat16

    x = np.asarray(x)
    xTf = np.ascontiguousarray(x.reshape(S, HID).T).astype(bf16)   # [HID, S]
    wqkvT = np.ascontiguousarray(np.asarray(W_qkv).T).astype(bf16)
    woutT = np.ascontiguousarray(np.asarray(W_out).T).astype(bf16)

    in_maps = []
    for c in range(NCORES):
        in_maps.append({
            "xT": np.ascontiguousarray(xTf[:, c * SQ:(c + 1) * SQ]),
            "wqkvT": wqkvT,
            "woutT": woutT,
        })
    res = run_bass_kernel_spmd(nc, in_maps, core_ids=list(range(NCORES)),
                               trace=_trace)
    out = np.concatenate([res.results[c]["y"] for c in range(NCORES)],
                         axis=0)
    out = out.reshape(1, S, HID).astype(np.float32)
    if _trace:
        kernel.last_results = res
    return out


# revision 30
# speedup vs baseline: 1.0230x; 1.0230x over previous
"""Distributed ring-attention kernel for Trainium2 (8 NeuronCores, Bass/Tile).

Strategy (seq-parallel attention, full softmax without max-subtraction):
  - Host: transpose/cast inputs to bf16; shard x.T column-wise (seq) across 8 cores.
  - Per core: project Q/K/V for its 512-seq shard; AllGather K^T and V
    across cores; compute full attention for its Q shard over the whole
    4096-length K/V; out-projection; write its y shard.
  - Collective schedule: ncfw boots ~21us in and runs a ~37us comm-init
    barrier, so no gathered data can land before ~85us. The K/V exchange is
    4 combined AllGathers of 2 head-pairs each ([kt | v] packed in one flat
    buffer per op), sized so the supply stays just ahead of the two exp
    engines' combined consumption (~0.22M elem/us).
  - Scores are computed transposed (S^T = K @ Q^T, kpos on partitions) so the
    exp'd probabilities feed the P@V matmul directly as the stationary-side
    contraction; head pairs ride the PE array row groups (tile_position) so
    the two 64-deep score matmuls run concurrently. Softmax denominator
    comes from a ones-column appended to V. Softmax skips max-subtraction:
    scores are O(1) here, so exp is numerically safe.
  - exp is SPLIT across two engines: ScalarE runs native exp on ~2/3 of the
    score groups; the DVE handles the rest with a Schraudolph-style bit
    trick: bf16_bits = int16(s * 128*log2e/8 + b), written through an int16
    bitcast of the bf16 tile (one tensor_scalar op per group). The constant
    bias of the trick cancels in softmax; only the sawtooth residual
    (~1.6e-2 worst case full-DVE, ~1.3e-2 at 1/3 share) remains.
"""

import numpy as np
import ml_dtypes

HID = 1024
HEADS = 16
HD = 64
S = 4096
NCORES = 8
SQ = S // NCORES          # 512 q rows per core
PAIRS = HEADS // 2        # 8 head pairs (128 rows of qkvT per pair)
KTILES = S // 128         # 32 kpos tiles per head
VAUG = HD + 1             # 65: V plus ones column
SCALE = 1.0 / np.sqrt(HD)

# Schraudolph exp in bf16-bit space: bits16 = trunc(a*s + b) approximates
# bf16(exp(s/8)) bit pattern.  a = 128*log2(e)*SCALE;  b = 128*(127-sigma)+0.5
SCH_A = 128.0 * 1.4426950408889634 * SCALE
SCH_B = 16245.48

_cache = {}


def _build():
    import concourse.bass as bass
    import concourse.mybir as mybir
    import concourse.tile as tile
    from concourse import bacc

    dt = mybir.dt
    nc = bacc.Bacc("TRN2", target_bir_lowering=False, debug=False,
                   num_devices=NCORES)

    xT = nc.dram_tensor("xT", [HID, SQ], dt.bfloat16, kind="ExternalInput").ap()
    wqkvT = nc.dram_tensor("wqkvT", [HID, 3 * HID], dt.bfloat16,
                           kind="ExternalInput").ap()
    woutT = nc.dram_tensor("woutT", [HID, HID], dt.bfloat16,
                           kind="ExternalInput").ap()
    y = nc.dram_tensor("y", [SQ, HID], dt.float32, kind="ExternalOutput").ap()

    with tile.TileContext(nc) as tc:
        _body(nc, tc, bass, mybir, xT, wqkvT, woutT, y)

    nc.compile()
    return nc


def _body(nc, tc, bass, mybir, xT, wqkvT, woutT, y):
    dt = mybir.dt
    f32, bf16 = dt.float32, dt.bfloat16
    RG = [list(range(NCORES))]

    with (
        tc.tile_pool(name="dram", bufs=1, space="DRAM") as dram,
        tc.tile_pool(name="resident", bufs=1) as res,
        tc.tile_pool(name="stream", bufs=1) as st,
    ):
        # ---- DRAM bounce buffers: one combined [kt... | v...] buffer per
        # UNIT of head pairs; first two units are single pairs so the
        # attention pipeline starts as early as possible, later units
        # amortize the ~16us fixed ncfw per-op cost.  V sections are staged
        # PRE-AUGMENTED ([q, hh, tl, 65] with the ones column included) so
        # the post-gather vah load is one contiguous-line DMA. ----
        UNITS = [[0], [1], [2, 3], [4, 5], [6, 7]]
        KSEC = 128 * SQ             # kt section elems, layout [r 128, q 512]
        VSEC = 128 * 2 * 4 * VAUG   # v section elems, layout [q, hh, tl, 65]
        unit_of = {}
        for u, prs in enumerate(UNITS):
            for i, p in enumerate(prs):
                unit_of[p] = (u, i)
        ktvb, ktvg = [], []
        for u, prs in enumerate(UNITS):
            # +16 pad elems at the end: a dependency-carrier byte can be
            # written there to delay a gather's trigger without corrupting
            # real sections
            usz = len(prs) * (KSEC + VSEC) + 16
            ktvb.append(dram.tile([1, usz], bf16, name=f"ktvb{u}"))
            ktvg.append(dram.tile([NCORES, usz], bf16, addr_space="Shared",
                                  name=f"ktvg{u}"))

        # ---- load xT (hidden x local-seq), 8 resident tiles ----
        xt = []
        for k in range(8):
            t = res.tile([128, SQ], bf16, tag=f"xt{k}", name=f"xt{k}")
            nc.sync.dma_start(t[:], xT[k * 128:(k + 1) * 128, :])
            xt.append(t)

        # dummy partition_broadcast at boot: forces the gpsimd ucode library
        # load (~15us) NOW instead of mid-attention, where it head-of-line
        # blocked the Vector queue for ~30us behind the first normalize
        dl0 = st.tile([1, 16], f32, tag="dl0", bufs=1)
        nc.vector.memset(dl0[:], 1.0)
        dlb = st.tile([2, 16], f32, tag="dlb", bufs=1)
        nc.gpsimd.partition_broadcast(dlb[:], dl0[:])

        # wqkvT strip views for batched weight loads
        wq4 = wqkvT.rearrange("(k p) (m c) -> p m k c", p=128, c=128)
        wv2 = wqkvT.rearrange("(k p) (m c) -> p m k c", p=128, c=128)

        def kt_proj(m, psP):
            """K^T rows for pair m (qkvT rows 1024+m*128) -> its unit's
            bounce buffer."""
            u, i = unit_of[m]
            ws = st.tile([128, 8 * 128], bf16, tag="wl", bufs=4)
            nc.sync.dma_start(ws.rearrange("p (k c) -> p k c", c=128),
                              wq4[:, 8 + m, :, :])
            ps = psP.tile([128, SQ], f32, tag="proj", bufs=4)
            for k in range(8):
                nc.tensor.matmul(ps[:], ws[:, k * 128:(k + 1) * 128],
                                 xt[k][:], start=(k == 0), stop=(k == 7))
            sb = st.tile([128, SQ], bf16, tag="kt_stage", bufs=4)
            nc.vector.tensor_copy(sb[:], ps[:])
            dst = ktvb[u][0, i * KSEC:(i + 1) * KSEC]
            nc.sync.dma_start(dst.rearrange("(r q) -> r q", q=SQ), sb[:])

        def v_proj(u, psP):
            """V rows (natural [s, (i hh d)]) for unit u's pairs -> bounce.

            One psum group per s-tile of 128; output n*128 wide."""
            prs = UNITS[u]
            n = len(prs)
            wvs = st.tile([128, 8 * n * 128], bf16, tag="wvs", bufs=2)
            wvs3 = wvs.rearrange("p (k c) -> p k c", c=n * 128)
            nc.sync.dma_start(
                wvs3.rearrange("p k (pr c) -> p k pr c", c=128),
                wv2[:, 16 + prs[0]:16 + prs[0] + n, :, :].rearrange(
                    "p pr k c -> p k pr c"))
            vbase = n * KSEC
            for sti in range(4):
                ps = psP.tile([128, n * 128], f32, tag="proj", bufs=4)
                for k in range(8):
                    nc.tensor.matmul(
                        ps[:], xt[k][:, sti * 128:(sti + 1) * 128],
                        wvs[:, k * n * 128:(k + 1) * n * 128],
                        start=(k == 0), stop=(k == 7))
                # stage augmented [q, i, hh, 65] with the ones column so the
                # gathered buffer is directly vah-shaped
                sb = st.tile([128, n * 2 * VAUG], bf16, tag="kv_stage",
                             bufs=4)
                sb4 = sb.rearrange("q (i hh v) -> q i hh v", hh=2, v=VAUG)
                nc.vector.tensor_copy(
                    sb4[:, :, :, 0:HD],
                    ps.rearrange("q (i hh d) -> q i hh d", hh=2, d=HD))
                nc.vector.memset(sb4[:, :, :, HD], 1.0)
                for i in range(n):
                    vpart = ktvb[u][0, vbase + i * VSEC:
                                    vbase + (i + 1) * VSEC].rearrange(
                        "(q hh tl v) -> q hh tl v", q=128, hh=2, v=VAUG)
                    nc.sync.dma_start(vpart[:, :, sti, :], sb4[:, i, :, :])
            nc.gpsimd.collective_compute(
                "AllGather", mybir.AluOpType.bypass, replica_groups=RG,
                ins=[ktvb[u].opt()], outs=[ktvg[u].opt()])

        qt = [None] * PAIRS

        def q_proj(m, psP):
            ws = st.tile([128, 8 * 128], bf16, tag="wl", bufs=4)
            nc.sync.dma_start(ws.rearrange("p (k c) -> p k c", c=128),
                              wq4[:, m, :, :])
            ps = psP.tile([128, SQ], f32, tag="proj", bufs=4)
            for k in range(8):
                nc.tensor.matmul(ps[:], ws[:, k * 128:(k + 1) * 128],
                                 xt[k][:], start=(k == 0), stop=(k == 7))
            t = res.tile([128, SQ], bf16, tag=f"qt{m}", name=f"qt{m}")
            nc.vector.tensor_copy(t[:], ps[:])
            qt[m] = t

        with tc.tile_pool(name="psP", bufs=1, space="PSUM") as psP:
            # per unit: stage kt for both pairs, then v, then fire the
            # combined AllGather; collectives run serially on gpsimd so the
            # emission order is the wire order.  q_proj 2..7 are deferred
            # into the gather window to keep the PE warm (HAM throttle).
            for u in range(len(UNITS)):
                for p in UNITS[u]:
                    kt_proj(p, psP)
                v_proj(u, psP)
            q_proj(0, psP)
            q_proj(1, psP)

        # ---- attention (head pairs row-packed on the PE array) ----
        attn = []
        for p in range(PAIRS):
            t = res.tile([128, SQ], bf16, tag=f"attn{p}", name=f"attn{p}")
            attn.append(t)

        # out-projection weights (pair-stacked rows: odd heads at
        # partitions 64..127)
        wo5 = woutT.rearrange("(pp r) (o c) -> r o pp c", r=128, c=512)
        wo = []
        for och in range(2):
            w = res.tile([128, PAIRS * 512], bf16, tag=f"wo{och}",
                         name=f"wo{och}")
            nc.sync.dma_start(
                w.rearrange("r (pp c) -> r pp c", c=512), wo5[:, och])
            wo.append(w)

        # exp engine split: every 3rd 2-slot group goes to the DVE via the
        # Schraudolph bit trick (int16 write into the bf16 tile)
        gctr = [0]

        def emit_exp(pt, sc, gw):
            g = gctr[0]
            gctr[0] += 1
            if g % 4 == 3:
                nc.vector.tensor_scalar(
                    pt[:, 0:gw].bitcast(mybir.dt.int16), sc[:, 0:gw],
                    float(SCH_A), float(SCH_B),
                    mybir.AluOpType.mult, mybir.AluOpType.add)
            else:
                nc.scalar.activation(pt[:, 0:gw], sc[:, 0:gw],
                                     mybir.ActivationFunctionType.Exp,
                                     scale=float(SCALE))

        # deferred q projections run in the gather window (own pool scope,
        # sequential with psP/psA)
        with tc.tile_pool(name="psQ", bufs=1, space="PSUM") as psQ:
            for m in range(2, PAIRS):
                q_proj(m, psQ)

        def emit_loads(p):
            """Issue pair p's K^T strip + augmented-V loads from the
            gathered buffers (one contiguous-line DMA each).  Queue
            assignment alternates by pair parity so a prefetched load
            waiting on the next AllGather never head-of-line-blocks the
            previous pair's loads."""
            u, i = unit_of[p]
            n = len(UNITS[u])
            eng = nc.sync if p % 2 == 0 else nc.gpsimd
            ktg3 = ktvg[u][:, i * KSEC:(i + 1) * KSEC].rearrange(
                "c (r q) -> r c q", q=SQ)
            kth = st.tile([128, S], bf16, tag="kth", bufs=3)
            eng.dma_start(kth.rearrange("r (c q) -> r c q", q=SQ), ktg3)
            vbase = n * KSEC + i * VSEC
            vgv = ktvg[u][:, vbase:vbase + VSEC].rearrange(
                "c (q hh tv) -> hh q c tv", q=128, hh=2, tv=4 * VAUG)
            vah = []
            for e in range(2):
                va = st.tile([128, KTILES * VAUG], bf16, tag="vah", bufs=6)
                eng.dma_start(va.rearrange("q (c tv) -> q c tv",
                                           tv=4 * VAUG), vgv[e])
                vah.append(va)
            return kth, vah

        # prefetch depth 1: loads for pair p+1 are emitted at pair p's top.
        # Depth 2 head-of-line-blocked the DMA queues: a prefetched load
        # waiting on a not-yet-landed AllGather stalls every DMA behind it.
        #
        # The whole attention runs as ONE global stream of 2-slot groups
        # with the PV matmuls lagging RA groups behind the scores ACROSS
        # pair boundaries, so the PE never drains a PV-only flush tail
        # between pairs (that bubble cost ~3us x 8 pairs).
        RA = 5
        pending_norm = []
        loaded = {}
        GPP = KTILES * 2 // 2       # 32 groups per pair
        sched = [(p, g) for p in range(PAIRS) for g in range(GPP)]
        state = {}

        def emit_scores(p, g):
            if g == 0:
                if p + 1 < PAIRS:
                    loaded[p + 1] = emit_loads(p + 1)
                kth, vah = loaded.pop(p)
                pv = [psA.tile([128, 512], f32, tag="pv", bufs=2,
                               name=f"pv{p}_{e}") for e in range(2)]
                state[p] = (kth, vah, pv, [])
            if g == GPP // 2 and pending_norm:
                # flush the previous pair's deferred divide chain mid-pair
                # so it never lands in the post-attention tail
                pending_norm.pop()()
            kth, vah, pv, pts = state[p]
            sc = psA.tile([128, 1024], f32, tag="sc", bufs=3)
            for idx, e in enumerate((0, 1)):
                nc.tensor.matmul(
                    sc[:, idx * 512:(idx + 1) * 512],
                    kth[e * 64:(e + 1) * 64, g * 128:(g + 1) * 128],
                    qt[p][e * 64:(e + 1) * 64, :],
                    start=True, stop=True,
                    tile_position=(e * 64, 0))
            pt = st.tile([128, 1024], bf16, tag="pt", bufs=RA + 2)
            emit_exp(pt, sc, 1024)
            pts.append(pt)

        def emit_pv(p, g):
            kth, vah, pv, pts = state[p]
            pt = pts[g]
            for idx, e in enumerate((0, 1)):
                nc.tensor.matmul(
                    pv[e][0:VAUG, :],
                    vah[e][:, g * VAUG:(g + 1) * VAUG],
                    pt[:, idx * 512:(idx + 1) * 512],
                    start=(g == 0), stop=(g == KTILES - 1))
            if g == KTILES - 1:
                finish_pair(p)

        def finish_pair(p):
            # Evacuate pv to SBUF right away so the PSUM slots free for the
            # next pair; DEFER the divide chain's emission by one pair so
            # its cross-engine round trips (gpsimd broadcast -> DVE recip)
            # never sit at the head of the Vector queue blocking exps.
            kth, vah, pv, pts = state.pop(p)
            pvs2 = []
            for e in range(2):
                pvs = st.tile([VAUG, 512], f32, tag="pvs", bufs=4)
                nc.vector.tensor_copy(pvs[:], pv[e][0:VAUG, :])
                pvs2.append(pvs)

            def emit_norm(p=p, pvs2=pvs2):
                neng = nc.sync if p % 2 == 0 else nc.gpsimd
                for e in range(2):
                    pvs = pvs2[e]
                    l0 = st.tile([1, 512], f32, tag="l0", bufs=2)
                    neng.dma_start(l0[:], pvs[64:65, :])
                    lb = st.tile([64, 512], f32, tag="lb", bufs=2)
                    nc.gpsimd.partition_broadcast(lb[:], l0[:])
                    rb = st.tile([64, 512], f32, tag="rb", bufs=2)
                    nc.vector.reciprocal_approx_fast(rb[:], lb[:])
                    if e == 0:
                        nc.vector.tensor_mul(attn[p][0:64, :],
                                             pvs[0:64, :], rb[:])
                    else:
                        ao = st.tile([64, SQ], bf16, tag="ao", bufs=2)
                        nc.vector.tensor_mul(ao[:], pvs[0:64, :], rb[:])
                        neng.dma_start(attn[p][64:128, :], ao[:])

            if pending_norm:
                pending_norm.pop()()
            pending_norm.append(emit_norm)

        with tc.tile_pool(name="psA", bufs=1, space="PSUM") as psA:
            loaded[0] = emit_loads(0)
            for gi, (p, g) in enumerate(sched):
                emit_scores(p, g)
                if gi >= RA:
                    emit_pv(*sched[gi - RA])
            for gi in range(len(sched) - RA, len(sched)):
                emit_pv(*sched[gi])

        for fn in pending_norm:
            fn()
        pending_norm.clear()

        # ---- out projection: y[s, o] = sum_h attn_h^T.T @ woutT[h rows] ----
        with tc.tile_pool(name="psY", bufs=1, space="PSUM") as psY:
            for sti in range(4):
                for och in range(2):
                    psa = psY.tile([128, 512], f32, tag="ya", bufs=4)
                    for p in range(PAIRS):
                        nc.tensor.matmul(
                            psa[:], attn[p][:, sti * 128:(sti + 1) * 128],
                            wo[och][:, p * 512:(p + 1) * 512],
                            start=(p == 0), stop=(p == PAIRS - 1))
                    ysb = st.tile([128, 512], f32, tag="ysb", bufs=4)
                    nc.vector.tensor_copy(ysb[:], psa[:])
                    nc.sync.dma_start(
                        y[sti * 128:(sti + 1) * 128,
                          och * 512:(och + 1) * 512], ysb[:])


def _get_nc():
    if "nc" not in _cache:
        _cache["nc"] = _build()
    return _cache["nc"]


def kernel(x, W_qkv, W_out, _trace=False):
    from concourse.bass_utils import run_bass_kernel_spmd

    nc = _get_nc()
    bf16 = ml_dtypes.bfloat16

    x = np.asarray(x)
    xTf = np.ascontiguousarray(x.reshape(S, HID).T).astype(bf16)   # [HID, S]
    wqkvT = np.ascontiguousarray(np.asarray(W_qkv).T).astype(bf16)
    woutT = np.ascontiguousarray(np.asarray(W_out).T).astype(bf16)

    in_maps = []
    for c in range(NCORES):
        in_maps.append({
            "xT": np.ascontiguousarray(xTf[:, c * SQ:(c + 1) * SQ]),
            "wqkvT": wqkvT,
            "woutT": woutT,
        })
    res = run_bass_kernel_spmd(nc, in_maps, core_ids=list(range(NCORES)),
                               trace=_trace)
    out = np.concatenate([res.results[c]["y"] for c in range(NCORES)],
                         axis=0)
    out = out.reshape(1, S, HID).astype(np.float32)
    if _trace:
        kernel.last_results = res
    return out


# revision 36
# speedup vs baseline: 1.0421x; 1.0187x over previous
"""Distributed ring-attention kernel for Trainium2 (8 NeuronCores, Bass/Tile).

Strategy (seq-parallel attention, full softmax without max-subtraction):
  - Host: transpose/cast inputs to bf16; shard x.T column-wise (seq) across 8 cores.
  - Per core: project Q/K/V for its 512-seq shard; AllGather K^T and V
    across cores; compute full attention for its Q shard over the whole
    4096-length K/V; out-projection; write its y shard.
  - Collective schedule: ncfw boots ~21us in and runs a ~37us comm-init
    barrier, so no gathered data can land before ~85us. The K/V exchange is
    4 combined AllGathers of 2 head-pairs each ([kt | v] packed in one flat
    buffer per op), sized so the supply stays just ahead of the two exp
    engines' combined consumption (~0.22M elem/us).
  - Scores are computed transposed (S^T = K @ Q^T, kpos on partitions) so the
    exp'd probabilities feed the P@V matmul directly as the stationary-side
    contraction; head pairs ride the PE array row groups (tile_position) so
    the two 64-deep score matmuls run concurrently. Softmax denominator
    comes from a ones-column appended to V. Softmax skips max-subtraction:
    scores are O(1) here, so exp is numerically safe.
  - exp is SPLIT across two engines: ScalarE runs native exp on ~2/3 of the
    score groups; the DVE handles the rest with a Schraudolph-style bit
    trick: bf16_bits = int16(s * 128*log2e/8 + b), written through an int16
    bitcast of the bf16 tile (one tensor_scalar op per group). The constant
    bias of the trick cancels in softmax; only the sawtooth residual
    (~1.6e-2 worst case full-DVE, ~1.3e-2 at 1/3 share) remains.
"""

import numpy as np
import ml_dtypes

HID = 1024
HEADS = 16
HD = 64
S = 4096
NCORES = 8
SQ = S // NCORES          # 512 q rows per core
PAIRS = HEADS // 2        # 8 head pairs (128 rows of qkvT per pair)
KTILES = S // 128         # 32 kpos tiles per head
VAUG = HD + 1             # 65: V plus ones column
SCALE = 1.0 / np.sqrt(HD)

# Schraudolph exp in bf16-bit space: bits16 = trunc(a*s + b) approximates
# bf16(exp(s/8)) bit pattern.  a = 128*log2(e)*SCALE;  b = 128*(127-sigma)+0.5
SCH_A = 128.0 * 1.4426950408889634 * SCALE
SCH_B = 16245.48

_cache = {}


def _build():
    import concourse.bass as bass
    import concourse.mybir as mybir
    import concourse.tile as tile
    from concourse import bacc

    dt = mybir.dt
    nc = bacc.Bacc("TRN2", target_bir_lowering=False, debug=False,
                   num_devices=NCORES)

    xT = nc.dram_tensor("xT", [HID, SQ], dt.bfloat16, kind="ExternalInput").ap()
    wqkvT = nc.dram_tensor("wqkvT", [HID, 3 * HID], dt.bfloat16,
                           kind="ExternalInput").ap()
    woutT = nc.dram_tensor("woutT", [HID, HID], dt.bfloat16,
                           kind="ExternalInput").ap()
    y = nc.dram_tensor("y", [SQ, HID], dt.float32, kind="ExternalOutput").ap()

    with tile.TileContext(nc) as tc:
        _body(nc, tc, bass, mybir, xT, wqkvT, woutT, y)

    nc.compile()
    return nc


def _body(nc, tc, bass, mybir, xT, wqkvT, woutT, y):
    dt = mybir.dt
    f32, bf16 = dt.float32, dt.bfloat16
    RG = [list(range(NCORES))]

    with (
        tc.tile_pool(name="dram", bufs=1, space="DRAM") as dram,
        tc.tile_pool(name="resident", bufs=1) as res,
        tc.tile_pool(name="stream", bufs=1) as st,
    ):
        # ---- DRAM bounce buffers: one combined [kt... | v...] buffer per
        # UNIT of head pairs; first two units are single pairs so the
        # attention pipeline starts as early as possible, later units
        # amortize the ~16us fixed ncfw per-op cost.  V sections are staged
        # PRE-AUGMENTED ([q, hh, tl, 65] with the ones column included) so
        # the post-gather vah load is one contiguous-line DMA. ----
        UNITS = [[0], [1], [2, 3], [4, 5], [6, 7]]
        KSEC = 128 * SQ             # kt section elems, layout [r 128, q 512]
        VSEC = 128 * 2 * 4 * VAUG   # v section elems, layout [q, hh, tl, 65]
        unit_of = {}
        for u, prs in enumerate(UNITS):
            for i, p in enumerate(prs):
                unit_of[p] = (u, i)
        ktvb, ktvg = [], []
        for u, prs in enumerate(UNITS):
            # +16 pad elems at the end: a dependency-carrier byte can be
            # written there to delay a gather's trigger without corrupting
            # real sections
            usz = len(prs) * (KSEC + VSEC) + 16
            ktvb.append(dram.tile([1, usz], bf16, name=f"ktvb{u}"))
            ktvg.append(dram.tile([NCORES, usz], bf16, addr_space="Shared",
                                  name=f"ktvg{u}"))

        # ---- load xT (hidden x local-seq), 8 resident tiles ----
        xt = []
        for k in range(8):
            t = res.tile([128, SQ], bf16, tag=f"xt{k}", name=f"xt{k}")
            nc.sync.dma_start(t[:], xT[k * 128:(k + 1) * 128, :])
            xt.append(t)

        # dummy partition_broadcast at boot: forces the gpsimd ucode library
        # load (~15us) NOW instead of mid-attention, where it head-of-line
        # blocked the Vector queue for ~30us behind the first normalize
        dl0 = st.tile([1, 16], f32, tag="dl0", bufs=1)
        nc.vector.memset(dl0[:], 1.0)
        dlb = st.tile([2, 16], f32, tag="dlb", bufs=1)
        nc.gpsimd.partition_broadcast(dlb[:], dl0[:])

        # wqkvT strip views for batched weight loads
        wq4 = wqkvT.rearrange("(k p) (m c) -> p m k c", p=128, c=128)
        wv2 = wqkvT.rearrange("(k p) (m c) -> p m k c", p=128, c=128)

        def kt_proj(m, psP):
            """K^T rows for pair m (qkvT rows 1024+m*128) -> its unit's
            bounce buffer."""
            u, i = unit_of[m]
            ws = st.tile([128, 8 * 128], bf16, tag="wl", bufs=4)
            nc.sync.dma_start(ws.rearrange("p (k c) -> p k c", c=128),
                              wq4[:, 8 + m, :, :])
            ps = psP.tile([128, SQ], f32, tag="proj", bufs=4)
            for k in range(8):
                nc.tensor.matmul(ps[:], ws[:, k * 128:(k + 1) * 128],
                                 xt[k][:], start=(k == 0), stop=(k == 7))
            sb = st.tile([128, SQ], bf16, tag="kt_stage", bufs=4)
            nc.vector.tensor_copy(sb[:], ps[:])
            dst = ktvb[u][0, i * KSEC:(i + 1) * KSEC]
            nc.sync.dma_start(dst.rearrange("(r q) -> r q", q=SQ), sb[:])

        def v_proj(u, psP):
            """V rows (natural [s, (i hh d)]) for unit u's pairs -> bounce.

            One psum group per s-tile of 128; output n*128 wide."""
            prs = UNITS[u]
            n = len(prs)
            wvs = st.tile([128, 8 * n * 128], bf16, tag="wvs", bufs=2)
            wvs3 = wvs.rearrange("p (k c) -> p k c", c=n * 128)
            nc.sync.dma_start(
                wvs3.rearrange("p k (pr c) -> p k pr c", c=128),
                wv2[:, 16 + prs[0]:16 + prs[0] + n, :, :].rearrange(
                    "p pr k c -> p k pr c"))
            vbase = n * KSEC
            for sti in range(4):
                ps = psP.tile([128, n * 128], f32, tag="proj", bufs=4)
                for k in range(8):
                    nc.tensor.matmul(
                        ps[:], xt[k][:, sti * 128:(sti + 1) * 128],
                        wvs[:, k * n * 128:(k + 1) * n * 128],
                        start=(k == 0), stop=(k == 7))
                # stage augmented [q, i, hh, 65] with the ones column so the
                # gathered buffer is directly vah-shaped
                sb = st.tile([128, n * 2 * VAUG], bf16, tag="kv_stage",
                             bufs=4)
                sb4 = sb.rearrange("q (i hh v) -> q i hh v", hh=2, v=VAUG)
                nc.vector.tensor_copy(
                    sb4[:, :, :, 0:HD],
                    ps.rearrange("q (i hh d) -> q i hh d", hh=2, d=HD))
                nc.vector.memset(sb4[:, :, :, HD], 1.0)
                for i in range(n):
                    vpart = ktvb[u][0, vbase + i * VSEC:
                                    vbase + (i + 1) * VSEC].rearrange(
                        "(q hh tl v) -> q hh tl v", q=128, hh=2, v=VAUG)
                    nc.sync.dma_start(vpart[:, :, sti, :], sb4[:, i, :, :])
            nc.gpsimd.collective_compute(
                "AllGather", mybir.AluOpType.bypass, replica_groups=RG,
                ins=[ktvb[u].opt()], outs=[ktvg[u].opt()])

        qt = [None] * PAIRS

        def q_proj(m, psP):
            ws = st.tile([128, 8 * 128], bf16, tag="wl", bufs=4)
            nc.sync.dma_start(ws.rearrange("p (k c) -> p k c", c=128),
                              wq4[:, m, :, :])
            ps = psP.tile([128, SQ], f32, tag="proj", bufs=4)
            for k in range(8):
                nc.tensor.matmul(ps[:], ws[:, k * 128:(k + 1) * 128],
                                 xt[k][:], start=(k == 0), stop=(k == 7))
            t = res.tile([128, SQ], bf16, tag=f"qt{m}", name=f"qt{m}")
            nc.vector.tensor_copy(t[:], ps[:])
            qt[m] = t

        with tc.tile_pool(name="psP", bufs=1, space="PSUM") as psP:
            # per unit: stage kt for both pairs, then v, then fire the
            # combined AllGather; collectives run serially on gpsimd so the
            # emission order is the wire order.  q_proj 2..7 are deferred
            # into the gather window to keep the PE warm (HAM throttle).
            for u in range(len(UNITS)):
                for p in UNITS[u]:
                    kt_proj(p, psP)
                v_proj(u, psP)
            q_proj(0, psP)
            q_proj(1, psP)

        # ---- attention (head pairs row-packed on the PE array) ----
        attn = []
        for p in range(PAIRS):
            t = res.tile([128, SQ], bf16, tag=f"attn{p}", name=f"attn{p}")
            attn.append(t)



        # exp engine split: every 3rd 2-slot group goes to the DVE via the
        # Schraudolph bit trick (int16 write into the bf16 tile)
        gctr = [0]

        def emit_exp(pt, sc, gw):
            g = gctr[0]
            gctr[0] += 1
            if g % 4 == 3:
                nc.vector.tensor_scalar(
                    pt[:, 0:gw].bitcast(mybir.dt.int16), sc[:, 0:gw],
                    float(SCH_A), float(SCH_B),
                    mybir.AluOpType.mult, mybir.AluOpType.add)
            else:
                nc.scalar.activation(pt[:, 0:gw], sc[:, 0:gw],
                                     mybir.ActivationFunctionType.Exp,
                                     scale=float(SCALE))

        # deferred q projections run in the gather window (own pool scope,
        # sequential with psP/psA), PACED by small cross-engine ping-pong
        # delays (~2us each) so the PE sees activity through the whole
        # pre-AG0 hole and HAM doesn't re-throttle it to half clock
        with tc.tile_pool(name="psQ", bufs=1, space="PSUM") as psQ:
            wk = st.tile([1, 64], bf16, tag="wk", bufs=2)
            nc.vector.tensor_copy(wk[:], xt[0][0:1, 0:64])
            for m in range(2, PAIRS):
                q_proj(m, psQ)
                for _ in range(2):
                    ps = psQ.tile([128, SQ], f32, tag="proj", bufs=4)
                    nc.tensor.matmul(ps[0:1, 0:64], wk[0:1, 0:1],
                                     wk[0:1, 0:64], start=True, stop=True)
                    wk = st.tile([1, 64], bf16, tag="wk", bufs=2)
                    nc.vector.tensor_copy(wk[:], ps[0:1, 0:64])

        def emit_loads(p):
            """Issue pair p's K^T strip + augmented-V loads from the
            gathered buffers (one contiguous-line DMA each).  Queue
            assignment alternates by pair parity so a prefetched load
            waiting on the next AllGather never head-of-line-blocks the
            previous pair's loads."""
            u, i = unit_of[p]
            n = len(UNITS[u])
            eng = nc.sync if p % 2 == 0 else nc.gpsimd
            ktg3 = ktvg[u][:, i * KSEC:(i + 1) * KSEC].rearrange(
                "c (r q) -> r c q", q=SQ)
            kth = st.tile([128, S], bf16, tag="kth", bufs=3)
            kth3 = kth.rearrange("r (c q) -> r c q", q=SQ)
            # first c-section as its own small DMA so the pair's first
            # score tiles can start while the bulk still streams
            eng.dma_start(kth3[:, 0:1, :], ktg3[:, 0:1, :])
            eng.dma_start(kth3[:, 1:NCORES, :], ktg3[:, 1:NCORES, :])
            vbase = n * KSEC + i * VSEC
            vgv = ktvg[u][:, vbase:vbase + VSEC].rearrange(
                "c (q hh tv) -> hh q c tv", q=128, hh=2, tv=4 * VAUG)
            vah = []
            for e in range(2):
                va = st.tile([128, KTILES * VAUG], bf16, tag="vah", bufs=6)
                eng.dma_start(va.rearrange("q (c tv) -> q c tv",
                                           tv=4 * VAUG), vgv[e])
                vah.append(va)
            return kth, vah

        # prefetch depth 1: loads for pair p+1 are emitted at pair p's top.
        # Depth 2 head-of-line-blocked the DMA queues: a prefetched load
        # waiting on a not-yet-landed AllGather stalls every DMA behind it.
        #
        # The whole attention runs as ONE global stream of 2-slot groups
        # with the PV matmuls lagging RA groups behind the scores ACROSS
        # pair boundaries, so the PE never drains a PV-only flush tail
        # between pairs (that bubble cost ~3us x 8 pairs).
        RA = 5
        pending_norm = []
        loaded = {}
        GPP = KTILES * 2 // 2       # 32 groups per pair
        sched = [(p, g) for p in range(PAIRS) for g in range(GPP)]
        state = {}

        def emit_scores(p, g):
            if g == 0:
                if p + 1 < PAIRS:
                    loaded[p + 1] = emit_loads(p + 1)
                kth, vah = loaded.pop(p)
                pv = [psA.tile([128, 512], f32, tag="pv", bufs=2,
                               name=f"pv{p}_{e}") for e in range(2)]
                state[p] = (kth, vah, pv, [])
            if g == GPP // 2 and pending_norm:
                # flush the previous pair's deferred divide chain mid-pair
                # so it never lands in the post-attention tail
                pending_norm.pop()()
            kth, vah, pv, pts = state[p]
            sc = psA.tile([128, 1024], f32, tag="sc", bufs=3)
            for idx, e in enumerate((0, 1)):
                nc.tensor.matmul(
                    sc[:, idx * 512:(idx + 1) * 512],
                    kth[e * 64:(e + 1) * 64, g * 128:(g + 1) * 128],
                    qt[p][e * 64:(e + 1) * 64, :],
                    start=True, stop=True,
                    tile_position=(e * 64, 0))
            pt = st.tile([128, 1024], bf16, tag="pt", bufs=RA + 2)
            emit_exp(pt, sc, 1024)
            pts.append(pt)

        def emit_pv(p, g):
            kth, vah, pv, pts = state[p]
            pt = pts[g]
            for idx, e in enumerate((0, 1)):
                nc.tensor.matmul(
                    pv[e][0:VAUG, :],
                    vah[e][:, g * VAUG:(g + 1) * VAUG],
                    pt[:, idx * 512:(idx + 1) * 512],
                    start=(g == 0), stop=(g == KTILES - 1))
            if g == KTILES - 1:
                finish_pair(p)

        def finish_pair(p):
            # Evacuate pv to SBUF right away so the PSUM slots free for the
            # next pair; DEFER the divide chain's emission by one pair so
            # its cross-engine round trips (gpsimd broadcast -> DVE recip)
            # never sit at the head of the Vector queue blocking exps.
            kth, vah, pv, pts = state.pop(p)
            pvs2 = []
            for e in range(2):
                pvs = st.tile([VAUG, 512], f32, tag="pvs", bufs=4)
                nc.vector.tensor_copy(pvs[:], pv[e][0:VAUG, :])
                pvs2.append(pvs)

            def emit_norm(p=p, pvs2=pvs2):
                neng = nc.sync if p % 2 == 0 else nc.gpsimd
                for e in range(2):
                    pvs = pvs2[e]
                    l0 = st.tile([1, 512], f32, tag="l0", bufs=2)
                    neng.dma_start(l0[:], pvs[64:65, :])
                    lb = st.tile([64, 512], f32, tag="lb", bufs=2)
                    nc.gpsimd.partition_broadcast(lb[:], l0[:])
                    rb = st.tile([64, 512], f32, tag="rb", bufs=2)
                    nc.vector.reciprocal_approx_fast(rb[:], lb[:])
                    if e == 0:
                        nc.vector.tensor_mul(attn[p][0:64, :],
                                             pvs[0:64, :], rb[:])
                    else:
                        ao = st.tile([64, SQ], bf16, tag="ao", bufs=2)
                        nc.vector.tensor_mul(ao[:], pvs[0:64, :], rb[:])
                        neng.dma_start(attn[p][64:128, :], ao[:])

            if pending_norm:
                pending_norm.pop()()
            pending_norm.append(emit_norm)

        with tc.tile_pool(name="psA", bufs=1, space="PSUM") as psA:
            loaded[0] = emit_loads(0)
            for gi, (p, g) in enumerate(sched):
                emit_scores(p, g)
                if gi >= RA:
                    emit_pv(*sched[gi - RA])
            for gi in range(len(sched) - RA, len(sched)):
                emit_pv(*sched[gi])

        for fn in pending_norm:
            fn()
        pending_norm.clear()

        # ---- out projection: y[s, o] = sum_h attn_h^T.T @ woutT[h rows];
        # 1024-wide bf16 moving operand covers both och blocks per chain ----
        wob = st.tile([128, PAIRS * 1024], bf16, tag="wob", name="wob")
        nc.sync.dma_start(
            wob.rearrange("r (pp oc) -> r pp oc", oc=1024),
            woutT.rearrange("(pp r) oc -> r pp oc", r=128))
        with tc.tile_pool(name="psY", bufs=1, space="PSUM") as psY:
            for sti in range(4):
                for och in range(2):
                    psa = psY.tile([128, 512], f32, tag="ya", bufs=4)
                    for p in range(PAIRS):
                        nc.tensor.matmul(
                            psa[:], attn[p][:, sti * 128:(sti + 1) * 128],
                            wob[:, p * 1024 + och * 512:
                                 p * 1024 + (och + 1) * 512],
                            start=(p == 0), stop=(p == PAIRS - 1))
                    ysb = st.tile([128, 512], f32, tag="ysb", bufs=4)
                    nc.vector.tensor_copy(ysb[:], psa[:])
                    nc.sync.dma_start(
                        y[sti * 128:(sti + 1) * 128,
                          och * 512:(och + 1) * 512], ysb[:])


def _get_nc():
    if "nc" not in _cache:
        _cache["nc"] = _build()
    return _cache["nc"]


def kernel(x, W_qkv, W_out, _trace=False):
    from concourse.bass_utils import run_bass_kernel_spmd

    nc = _get_nc()
    bf16 = ml_dtypes.bfloat16

    x = np.asarray(x)
    xTf = np.ascontiguousarray(x.reshape(S, HID).T).astype(bf16)   # [HID, S]
    wqkvT = np.ascontiguousarray(np.asarray(W_qkv).T).astype(bf16)
    woutT = np.ascontiguousarray(np.asarray(W_out).T).astype(bf16)

    in_maps = []
    for c in range(NCORES):
        in_maps.append({
            "xT": np.ascontiguousarray(xTf[:, c * SQ:(c + 1) * SQ]),
            "wqkvT": wqkvT,
            "woutT": woutT,
        })
    res = run_bass_kernel_spmd(nc, in_maps, core_ids=list(range(NCORES)),
                               trace=_trace)
    out = np.concatenate([res.results[c]["y"] for c in range(NCORES)],
                         axis=0)
    out = out.reshape(1, S, HID).astype(np.float32)
    if _trace:
        kernel.last_results = res
    return out


# revision 37
# speedup vs baseline: 1.0456x; 1.0033x over previous
"""Distributed ring-attention kernel for Trainium2 (8 NeuronCores, Bass/Tile).

Strategy (seq-parallel attention, full softmax without max-subtraction):
  - Host: transpose/cast inputs to bf16; shard x.T column-wise (seq) across 8 cores.
  - Per core: project Q/K/V for its 512-seq shard; AllGather K^T and V
    across cores; compute full attention for its Q shard over the whole
    4096-length K/V; out-projection; write its y shard.
  - Collective schedule: ncfw's comm-init barrier + first-op warmup cost a
    near-constant ~90-115us from boot to first gathered data (the barrier
    and AG0 durations trade off run to run), so the whole attention
    pipeline starts ~125-145us in.  The K/V exchange is 5 AllGathers over
    units [1,1,2,2,2] head pairs ([kt... | v...] packed flat per op).  V is
    staged PRE-AUGMENTED ([q, hh, tl, 65] with the softmax ones-column
    included) so each post-gather vah load is one contiguous-line DMA.
    Per-pair loads alternate sync/gpsimd DMA queues (a prefetched load
    waiting on the next AllGather must not head-of-line-block the previous
    pair's loads), with prefetch depth 1.
  - Scores are computed transposed (S^T = K @ Q^T, kpos on partitions) so the
    exp'd probabilities feed the P@V matmul directly as the stationary-side
    contraction; head pairs ride the PE array row groups (tile_position) so
    the two 64-deep score matmuls run concurrently. Softmax denominator
    comes from a ones-column appended to V. Softmax skips max-subtraction:
    scores are O(1) here, so exp is numerically safe.
  - The attention is ONE global stream of 2-slot groups with the PV matmuls
    lagging RA=5 groups behind scores ACROSS pair boundaries (no PV-flush
    bubble between pairs).  Each pair's divide chain (gpsimd broadcast ->
    DVE recip/mul) is deferred into the middle of the NEXT pair so its
    cross-engine round trips never block the Vector queue (which carries
    the DVE exps); the gpsimd ucode library is pre-loaded at boot by a
    dummy partition_broadcast (the lazy load cost ~15us mid-attention).
  - exp is SPLIT across two engines: ScalarE runs native exp on 3/4 of the
    score groups; the DVE handles every 4th with a Schraudolph-style bit
    trick: bf16_bits = int16(s * 128*log2e/8 + b), written through an int16
    bitcast of the bf16 tile (one tensor_scalar op per group). The constant
    bias of the trick cancels in softmax; only the sawtooth residual
    remains (~1.1e-2 total at 1/4 share vs the 4.3e-3 all-ScalarE floor).
  - Deferred q projections are paced through the pre-AG0 hole with small
    cross-engine ping-pong delays so HAM never sees a >3.4us PE idle window
    and re-throttles the clock.
"""

import numpy as np
import ml_dtypes

HID = 1024
HEADS = 16
HD = 64
S = 4096
NCORES = 8
SQ = S // NCORES          # 512 q rows per core
PAIRS = HEADS // 2        # 8 head pairs (128 rows of qkvT per pair)
KTILES = S // 128         # 32 kpos tiles per head
VAUG = HD + 1             # 65: V plus ones column
SCALE = 1.0 / np.sqrt(HD)

# Schraudolph exp in bf16-bit space: bits16 = trunc(a*s + b) approximates
# bf16(exp(s/8)) bit pattern.  a = 128*log2(e)*SCALE;  b = 128*(127-sigma)+0.5
SCH_A = 128.0 * 1.4426950408889634 * SCALE
SCH_B = 16245.48

_cache = {}


def _build():
    import concourse.bass as bass
    import concourse.mybir as mybir
    import concourse.tile as tile
    from concourse import bacc

    dt = mybir.dt
    nc = bacc.Bacc("TRN2", target_bir_lowering=False, debug=False,
                   num_devices=NCORES)

    xT = nc.dram_tensor("xT", [HID, SQ], dt.bfloat16, kind="ExternalInput").ap()
    wqkvT = nc.dram_tensor("wqkvT", [HID, 3 * HID], dt.bfloat16,
                           kind="ExternalInput").ap()
    woutT = nc.dram_tensor("woutT", [HID, HID], dt.bfloat16,
                           kind="ExternalInput").ap()
    y = nc.dram_tensor("y", [SQ, HID], dt.float32, kind="ExternalOutput").ap()

    with tile.TileContext(nc) as tc:
        _body(nc, tc, bass, mybir, xT, wqkvT, woutT, y)

    nc.compile()
    return nc


def _body(nc, tc, bass, mybir, xT, wqkvT, woutT, y):
    dt = mybir.dt
    f32, bf16 = dt.float32, dt.bfloat16
    RG = [list(range(NCORES))]

    with (
        tc.tile_pool(name="dram", bufs=1, space="DRAM") as dram,
        tc.tile_pool(name="resident", bufs=1) as res,
        tc.tile_pool(name="stream", bufs=1) as st,
    ):
        # ---- DRAM bounce buffers: one combined [kt... | v...] buffer per
        # UNIT of head pairs; first two units are single pairs so the
        # attention pipeline starts as early as possible, later units
        # amortize the ~16us fixed ncfw per-op cost.  V sections are staged
        # PRE-AUGMENTED ([q, hh, tl, 65] with the ones column included) so
        # the post-gather vah load is one contiguous-line DMA. ----
        UNITS = [[0], [1], [2, 3], [4, 5], [6, 7]]
        KSEC = 128 * SQ             # kt section elems, layout [r 128, q 512]
        VSEC = 128 * 2 * 4 * VAUG   # v section elems, layout [q, hh, tl, 65]
        unit_of = {}
        for u, prs in enumerate(UNITS):
            for i, p in enumerate(prs):
                unit_of[p] = (u, i)
        ktvb, ktvg = [], []
        for u, prs in enumerate(UNITS):
            # +16 pad elems at the end: a dependency-carrier byte can be
            # written there to delay a gather's trigger without corrupting
            # real sections
            usz = len(prs) * (KSEC + VSEC) + 16
            ktvb.append(dram.tile([1, usz], bf16, name=f"ktvb{u}"))
            ktvg.append(dram.tile([NCORES, usz], bf16, addr_space="Shared",
                                  name=f"ktvg{u}"))

        # ---- load xT (hidden x local-seq), 8 resident tiles ----
        xt = []
        for k in range(8):
            t = res.tile([128, SQ], bf16, tag=f"xt{k}", name=f"xt{k}")
            nc.sync.dma_start(t[:], xT[k * 128:(k + 1) * 128, :])
            xt.append(t)

        # dummy partition_broadcast at boot: forces the gpsimd ucode library
        # load (~15us) NOW instead of mid-attention, where it head-of-line
        # blocked the Vector queue for ~30us behind the first normalize
        dl0 = st.tile([1, 16], f32, tag="dl0", bufs=1)
        nc.vector.memset(dl0[:], 1.0)
        dlb = st.tile([2, 16], f32, tag="dlb", bufs=1)
        nc.gpsimd.partition_broadcast(dlb[:], dl0[:])

        # wqkvT strip views for batched weight loads
        wq4 = wqkvT.rearrange("(k p) (m c) -> p m k c", p=128, c=128)
        wv2 = wqkvT.rearrange("(k p) (m c) -> p m k c", p=128, c=128)

        def kt_proj(m, psP):
            """K^T rows for pair m (qkvT rows 1024+m*128) -> its unit's
            bounce buffer."""
            u, i = unit_of[m]
            ws = st.tile([128, 8 * 128], bf16, tag="wl", bufs=4)
            nc.sync.dma_start(ws.rearrange("p (k c) -> p k c", c=128),
                              wq4[:, 8 + m, :, :])
            ps = psP.tile([128, SQ], f32, tag="proj", bufs=4)
            for k in range(8):
                nc.tensor.matmul(ps[:], ws[:, k * 128:(k + 1) * 128],
                                 xt[k][:], start=(k == 0), stop=(k == 7))
            sb = st.tile([128, SQ], bf16, tag="kt_stage", bufs=4)
            nc.vector.tensor_copy(sb[:], ps[:])
            dst = ktvb[u][0, i * KSEC:(i + 1) * KSEC]
            nc.sync.dma_start(dst.rearrange("(r q) -> r q", q=SQ), sb[:])

        def v_proj(u, psP):
            """V rows (natural [s, (i hh d)]) for unit u's pairs -> bounce.

            One psum group per s-tile of 128; output n*128 wide."""
            prs = UNITS[u]
            n = len(prs)
            wvs = st.tile([128, 8 * n * 128], bf16, tag="wvs", bufs=2)
            wvs3 = wvs.rearrange("p (k c) -> p k c", c=n * 128)
            nc.sync.dma_start(
                wvs3.rearrange("p k (pr c) -> p k pr c", c=128),
                wv2[:, 16 + prs[0]:16 + prs[0] + n, :, :].rearrange(
                    "p pr k c -> p k pr c"))
            vbase = n * KSEC
            for sti in range(4):
                ps = psP.tile([128, n * 128], f32, tag="proj", bufs=4)
                for k in range(8):
                    nc.tensor.matmul(
                        ps[:], xt[k][:, sti * 128:(sti + 1) * 128],
                        wvs[:, k * n * 128:(k + 1) * n * 128],
                        start=(k == 0), stop=(k == 7))
                # stage augmented [q, i, hh, 65] with the ones column so the
                # gathered buffer is directly vah-shaped
                sb = st.tile([128, n * 2 * VAUG], bf16, tag="kv_stage",
                             bufs=4)
                sb4 = sb.rearrange("q (i hh v) -> q i hh v", hh=2, v=VAUG)
                nc.vector.tensor_copy(
                    sb4[:, :, :, 0:HD],
                    ps.rearrange("q (i hh d) -> q i hh d", hh=2, d=HD))
                nc.vector.memset(sb4[:, :, :, HD], 1.0)
                for i in range(n):
                    vpart = ktvb[u][0, vbase + i * VSEC:
                                    vbase + (i + 1) * VSEC].rearrange(
                        "(q hh tl v) -> q hh tl v", q=128, hh=2, v=VAUG)
                    nc.sync.dma_start(vpart[:, :, sti, :], sb4[:, i, :, :])
            nc.gpsimd.collective_compute(
                "AllGather", mybir.AluOpType.bypass, replica_groups=RG,
                ins=[ktvb[u].opt()], outs=[ktvg[u].opt()])

        qt = [None] * PAIRS

        def q_proj(m, psP):
            ws = st.tile([128, 8 * 128], bf16, tag="wl", bufs=4)
            nc.sync.dma_start(ws.rearrange("p (k c) -> p k c", c=128),
                              wq4[:, m, :, :])
            ps = psP.tile([128, SQ], f32, tag="proj", bufs=4)
            for k in range(8):
                nc.tensor.matmul(ps[:], ws[:, k * 128:(k + 1) * 128],
                                 xt[k][:], start=(k == 0), stop=(k == 7))
            t = res.tile([128, SQ], bf16, tag=f"qt{m}", name=f"qt{m}")
            nc.vector.tensor_copy(t[:], ps[:])
            qt[m] = t

        with tc.tile_pool(name="psP", bufs=1, space="PSUM") as psP:
            # per unit: stage kt for both pairs, then v, then fire the
            # combined AllGather; collectives run serially on gpsimd so the
            # emission order is the wire order.  q_proj 2..7 are deferred
            # into the gather window to keep the PE warm (HAM throttle).
            for u in range(len(UNITS)):
                for p in UNITS[u]:
                    kt_proj(p, psP)
                v_proj(u, psP)
            q_proj(0, psP)
            q_proj(1, psP)

        # ---- attention (head pairs row-packed on the PE array) ----
        attn = []
        for p in range(PAIRS):
            t = res.tile([128, SQ], bf16, tag=f"attn{p}", name=f"attn{p}")
            attn.append(t)



        # exp engine split: every 3rd 2-slot group goes to the DVE via the
        # Schraudolph bit trick (int16 write into the bf16 tile)
        gctr = [0]

        def emit_exp(pt, sc, gw):
            g = gctr[0]
            gctr[0] += 1
            if g % 4 == 3:
                nc.vector.tensor_scalar(
                    pt[:, 0:gw].bitcast(mybir.dt.int16), sc[:, 0:gw],
                    float(SCH_A), float(SCH_B),
                    mybir.AluOpType.mult, mybir.AluOpType.add)
            else:
                nc.scalar.activation(pt[:, 0:gw], sc[:, 0:gw],
                                     mybir.ActivationFunctionType.Exp,
                                     scale=float(SCALE))

        # deferred q projections run in the gather window (own pool scope,
        # sequential with psP/psA), PACED by small cross-engine ping-pong
        # delays (~2us each) so the PE sees activity through the whole
        # pre-AG0 hole and HAM doesn't re-throttle it to half clock
        with tc.tile_pool(name="psQ", bufs=1, space="PSUM") as psQ:
            wk = st.tile([1, 64], bf16, tag="wk", bufs=2)
            nc.vector.tensor_copy(wk[:], xt[0][0:1, 0:64])
            for m in range(2, PAIRS):
                q_proj(m, psQ)
                for _ in range(2):
                    ps = psQ.tile([128, SQ], f32, tag="proj", bufs=4)
                    nc.tensor.matmul(ps[0:1, 0:64], wk[0:1, 0:1],
                                     wk[0:1, 0:64], start=True, stop=True)
                    wk = st.tile([1, 64], bf16, tag="wk", bufs=2)
                    nc.vector.tensor_copy(wk[:], ps[0:1, 0:64])

        def emit_loads(p):
            """Issue pair p's K^T strip + augmented-V loads from the
            gathered buffers (one contiguous-line DMA each).  Queue
            assignment alternates by pair parity so a prefetched load
            waiting on the next AllGather never head-of-line-blocks the
            previous pair's loads."""
            u, i = unit_of[p]
            n = len(UNITS[u])
            eng = nc.sync if p % 2 == 0 else nc.gpsimd
            ktg3 = ktvg[u][:, i * KSEC:(i + 1) * KSEC].rearrange(
                "c (r q) -> r c q", q=SQ)
            kth = st.tile([128, S], bf16, tag="kth", bufs=3)
            kth3 = kth.rearrange("r (c q) -> r c q", q=SQ)
            # first c-section as its own small DMA so the pair's first
            # score tiles can start while the bulk still streams
            eng.dma_start(kth3[:, 0:1, :], ktg3[:, 0:1, :])
            eng.dma_start(kth3[:, 1:NCORES, :], ktg3[:, 1:NCORES, :])
            vbase = n * KSEC + i * VSEC
            vgv = ktvg[u][:, vbase:vbase + VSEC].rearrange(
                "c (q hh tv) -> hh q c tv", q=128, hh=2, tv=4 * VAUG)
            vah = []
            for e in range(2):
                va = st.tile([128, KTILES * VAUG], bf16, tag="vah", bufs=6)
                eng.dma_start(va.rearrange("q (c tv) -> q c tv",
                                           tv=4 * VAUG), vgv[e])
                vah.append(va)
            return kth, vah

        # prefetch depth 1: loads for pair p+1 are emitted at pair p's top.
        # Depth 2 head-of-line-blocked the DMA queues: a prefetched load
        # waiting on a not-yet-landed AllGather stalls every DMA behind it.
        #
        # The whole attention runs as ONE global stream of 2-slot groups
        # with the PV matmuls lagging RA groups behind the scores ACROSS
        # pair boundaries, so the PE never drains a PV-only flush tail
        # between pairs (that bubble cost ~3us x 8 pairs).
        RA = 5
        pending_norm = []
        loaded = {}
        GPP = KTILES * 2 // 2       # 32 groups per pair
        sched = [(p, g) for p in range(PAIRS) for g in range(GPP)]
        state = {}

        def emit_scores(p, g):
            if g == 0:
                if p + 1 < PAIRS:
                    loaded[p + 1] = emit_loads(p + 1)
                kth, vah = loaded.pop(p)
                pv = [psA.tile([128, 512], f32, tag="pv", bufs=2,
                               name=f"pv{p}_{e}") for e in range(2)]
                state[p] = (kth, vah, pv, [])
            if g == GPP // 2 and pending_norm:
                # flush the previous pair's deferred divide chain mid-pair
                # so it never lands in the post-attention tail
                pending_norm.pop()()
            kth, vah, pv, pts = state[p]
            sc = psA.tile([128, 1024], f32, tag="sc", bufs=3)
            for idx, e in enumerate((0, 1)):
                nc.tensor.matmul(
                    sc[:, idx * 512:(idx + 1) * 512],
                    kth[e * 64:(e + 1) * 64, g * 128:(g + 1) * 128],
                    qt[p][e * 64:(e + 1) * 64, :],
                    start=True, stop=True,
                    tile_position=(e * 64, 0))
            pt = st.tile([128, 1024], bf16, tag="pt", bufs=RA + 2)
            emit_exp(pt, sc, 1024)
            pts.append(pt)

        def emit_pv(p, g):
            kth, vah, pv, pts = state[p]
            pt = pts[g]
            for idx, e in enumerate((0, 1)):
                nc.tensor.matmul(
                    pv[e][0:VAUG, :],
                    vah[e][:, g * VAUG:(g + 1) * VAUG],
                    pt[:, idx * 512:(idx + 1) * 512],
                    start=(g == 0), stop=(g == KTILES - 1))
            if g == KTILES - 1:
                finish_pair(p)

        def finish_pair(p):
            # Evacuate pv to SBUF right away so the PSUM slots free for the
            # next pair; DEFER the divide chain's emission by one pair so
            # its cross-engine round trips (gpsimd broadcast -> DVE recip)
            # never sit at the head of the Vector queue blocking exps.
            kth, vah, pv, pts = state.pop(p)
            pvs2 = []
            for e in range(2):
                pvs = st.tile([VAUG, 512], f32, tag="pvs", bufs=4)
                nc.vector.tensor_copy(pvs[:], pv[e][0:VAUG, :])
                pvs2.append(pvs)

            def emit_norm(p=p, pvs2=pvs2):
                neng = nc.sync if p % 2 == 0 else nc.gpsimd
                for e in range(2):
                    pvs = pvs2[e]
                    l0 = st.tile([1, 512], f32, tag="l0", bufs=2)
                    neng.dma_start(l0[:], pvs[64:65, :])
                    lb = st.tile([64, 512], f32, tag="lb", bufs=2)
                    nc.gpsimd.partition_broadcast(lb[:], l0[:])
                    rb = st.tile([64, 512], f32, tag="rb", bufs=2)
                    nc.vector.reciprocal_approx_fast(rb[:], lb[:])
                    if e == 0:
                        nc.vector.tensor_mul(attn[p][0:64, :],
                                             pvs[0:64, :], rb[:])
                    else:
                        ao = st.tile([64, SQ], bf16, tag="ao", bufs=2)
                        nc.vector.tensor_mul(ao[:], pvs[0:64, :], rb[:])
                        neng.dma_start(attn[p][64:128, :], ao[:])

            if pending_norm:
                pending_norm.pop()()
            pending_norm.append(emit_norm)

        with tc.tile_pool(name="psA", bufs=1, space="PSUM") as psA:
            loaded[0] = emit_loads(0)
            for gi, (p, g) in enumerate(sched):
                emit_scores(p, g)
                if gi >= RA:
                    emit_pv(*sched[gi - RA])
            for gi in range(len(sched) - RA, len(sched)):
                emit_pv(*sched[gi])

        for fn in pending_norm:
            fn()
        pending_norm.clear()

        # ---- out projection: y[s, o] = sum_h attn_h^T.T @ woutT[h rows];
        # 1024-wide bf16 moving operand covers both och blocks per chain ----
        wob = st.tile([128, PAIRS * 1024], bf16, tag="wob", name="wob")
        nc.sync.dma_start(
            wob.rearrange("r (pp oc) -> r pp oc", oc=1024),
            woutT.rearrange("(pp r) oc -> r pp oc", r=128))
        with tc.tile_pool(name="psY", bufs=1, space="PSUM") as psY:
            for sti in range(4):
                for och in range(2):
                    psa = psY.tile([128, 512], f32, tag="ya", bufs=4)
                    for p in range(PAIRS):
                        nc.tensor.matmul(
                            psa[:], attn[p][:, sti * 128:(sti + 1) * 128],
                            wob[:, p * 1024 + och * 512:
                                 p * 1024 + (och + 1) * 512],
                            start=(p == 0), stop=(p == PAIRS - 1))
                    ysb = st.tile([128, 512], f32, tag="ysb", bufs=4)
                    nc.vector.tensor_copy(ysb[:], psa[:])
                    nc.sync.dma_start(
                        y[sti * 128:(sti + 1) * 128,
                          och * 512:(och + 1) * 512], ysb[:])


def _get_nc():
    if "nc" not in _cache:
        _cache["nc"] = _build()
    return _cache["nc"]


def kernel(x, W_qkv, W_out, _trace=False):
    from concourse.bass_utils import run_bass_kernel_spmd

    nc = _get_nc()
    bf16 = ml_dtypes.bfloat16

    x = np.asarray(x)
    xTf = np.ascontiguousarray(x.reshape(S, HID).T).astype(bf16)   # [HID, S]
    wqkvT = np.ascontiguousarray(np.asarray(W_qkv).T).astype(bf16)
    woutT = np.ascontiguousarray(np.asarray(W_out).T).astype(bf16)

    in_maps = []
    for c in range(NCORES):
        in_maps.append({
            "xT": np.ascontiguousarray(xTf[:, c * SQ:(c + 1) * SQ]),
            "wqkvT": wqkvT,
            "woutT": woutT,
        })
    res = run_bass_kernel_spmd(nc, in_maps, core_ids=list(range(NCORES)),
                               trace=_trace)
    out = np.concatenate([res.results[c]["y"] for c in range(NCORES)],
                         axis=0)
    out = out.reshape(1, S, HID).astype(np.float32)
    if _trace:
        kernel.last_results = res
    return out
